# revision 1
# baseline (speedup 1.0000x reference)
"""Distributed kNN classifier (cosine sim, k=20, 9 classes) on 8 Trainium2 cores.

Strategy: shard the 100k-row train gallery across 8 cores (12500 rows each).
Host-side prep (free vs HW time): normalize train rows (folds the 1/||t||
cosine denominator into the data; 1/||x|| doesn't affect per-query ranking),
sort each shard by label and pad each class block to 256-row label-pure
segments (zero rows -> sim exactly 0, never in global top-20), transpose to
[D, N] layout for the PE.

Device per core: sims = x @ t_norm^T via PE matmuls accumulating in PSUM
(bf16 hi/lo 3-matmul trick for ~fp32 accuracy, or fp32r), then DVE InstMax
(top-8 per partition) per 256-col segment straight out of PSUM, level-2 merge
of the 58*4 segment candidates with 3 rounds of max/max_index/match_replace
-> per-core top-24 (value, position).

Host merge: 8*24=192 candidates per query, select global top-20 by value,
map positions -> labels via per-core segment tables, majority vote with
smallest-class tie-break (matches the reference's argmax).
"""

import os

import numpy as np

N_TRAIN = 100000
D = 256
N_TEST = 2048
K = 20
NUM_CLASSES = 9
N_CORES = 8
SHARD = N_TRAIN // N_CORES  # 12500

SEG = 512  # label-pure segment size = psum tile = matmul moving dim
QT = 128  # queries per tile
NQT = N_TEST // QT  # 16
L1_KEEP = 6  # candidates kept per segment (of the 8 InstMax returns)
TOPK_OUT = 24  # 3 rounds x 8
# segment count is adaptive: computed from the actual per-class padding at
# trace time (27 for balanced 12500-row shards), kernel cached per NSEG

MODE = os.environ.get("KNN_MODE", "bf16x3")  # bf16x3 | fp32r | fp32

_compiled = {}


def _build(mode, NSEG, NQT=NQT):
    import concourse.bacc as bacc
    import concourse.mybir as mybir
    import concourse.tile as tile

    N_PAD = NSEG * SEG
    N_TEST = NQT * QT
    NCAND = NSEG * L1_KEEP

    f32 = mybir.dt.float32
    bf16 = mybir.dt.bfloat16
    u32 = mybir.dt.uint32

    nc = bacc.Bacc(None, target_bir_lowering=False, debug=False)

    if mode == "bf16x3":
        in_dt = bf16
        t_hi = nc.dram_tensor("t_hi", [2, 128, N_PAD], in_dt, kind="ExternalInput")
        t_lo = nc.dram_tensor("t_lo", [2, 128, N_PAD], in_dt, kind="ExternalInput")
        x_hi = nc.dram_tensor("x_hi", [2, 128, N_TEST], in_dt, kind="ExternalInput")
        x_lo = nc.dram_tensor("x_lo", [2, 128, N_TEST], in_dt, kind="ExternalInput")
        t_drams, x_drams = [t_hi, t_lo], [x_hi, x_lo]
        # (x_hi+x_lo)@(t_hi+t_lo) ~= hi@hi + hi@lo + lo@hi
        terms = [(0, 0), (0, 1), (1, 0)]
    else:
        in_dt = f32
        t_full = nc.dram_tensor("t_full", [2, 128, N_PAD], in_dt, kind="ExternalInput")
        x_full = nc.dram_tensor("x_full", [2, 128, N_TEST], in_dt, kind="ExternalInput")
        t_drams, x_drams = [t_full], [x_full]
        terms = [(0, 0)]

    out_vals = nc.dram_tensor("out_vals", [NQT, 128, TOPK_OUT], f32, kind="ExternalOutput")
    out_pos = nc.dram_tensor("out_pos", [NQT, 128, TOPK_OUT], u32, kind="ExternalOutput")

    NEG = -3.0e38

    with tile.TileContext(nc) as tc:
        with (
            tc.tile_pool(name="wt", bufs=1) as wt_pool,
            tc.tile_pool(name="xt", bufs=1) as xt_pool,
            tc.tile_pool(name="cand", bufs=2) as cand_pool,
            tc.tile_pool(name="l2", bufs=2) as l2_pool,
            tc.tile_pool(name="outs", bufs=2) as out_pool,
            tc.tile_pool(name="psum", bufs=8, space="PSUM") as psum_pool,
        ):
            # resident SBUF copies of x and t (partition dim = contraction d')
            x_sb = [
                xt_pool.tile([128, 2, N_TEST], in_dt, tag=f"x{i}", name=f"x_sb{i}")
                for i in range(len(x_drams))
            ]
            for i, xd in enumerate(x_drams):
                for kk in range(2):
                    nc.sync.dma_start(out=x_sb[i][:, kk, :], in_=xd[kk])

            # t loaded in chunks so PE can start before the whole gallery lands
            NCHUNK = 8
            CH = N_PAD // NCHUNK  # 1856 = 3.625 segs... need seg-aligned: use 58/NCHUNK
            # chunk boundaries seg-aligned:
            seg_chunks = []
            per = (NSEG + NCHUNK - 1) // NCHUNK
            s0 = 0
            while s0 < NSEG:
                s1 = min(s0 + per, NSEG)
                seg_chunks.append((s0, s1))
                s0 = s1
            t_sb = [
                wt_pool.tile([128, 2, N_PAD], in_dt, tag=f"t{i}", name=f"t_sb{i}")
                for i in range(len(t_drams))
            ]
            for i, td in enumerate(t_drams):
                for kk in range(2):
                    for (s0, s1) in seg_chunks:
                        nc.sync.dma_start(
                            out=t_sb[i][:, kk, s0 * SEG : s1 * SEG],
                            in_=td[kk, :, s0 * SEG : s1 * SEG],
                        )

            cands = [
                cand_pool.tile([128, NSEG, 8], f32, tag=f"cand{qt}", name=f"cand{qt}")
                for qt in range(NQT)
            ]

            # ---- phase 1: matmul + per-segment top-8, segment outer ----
            for sp in range(NSEG):
                for qt in range(NQT):
                    ps = psum_pool.tile([128, SEG], f32, tag="ps")
                    nmm = len(terms) * 2
                    mi = 0
                    for (xi, ti) in terms:
                        for kk in range(2):
                            nc.tensor.matmul(
                                ps[:, :],
                                lhsT=x_sb[xi][:, kk, qt * QT : (qt + 1) * QT],
                                rhs=t_sb[ti][:, kk, sp * SEG : (sp + 1) * SEG],
                                start=(mi == 0),
                                stop=(mi == nmm - 1),
                            )
                            mi += 1
                    nc.vector.max(out=cands[qt][:, sp, :], in_=ps[:, :])

            # ---- phase 2: per-qtile level-2 merge ----
            for qt in range(NQT):
                work = l2_pool.tile([128, NCAND], f32, tag="work")
                nc.vector.tensor_copy(work[:, :], cands[qt][:, :, 0:L1_KEEP])
                vals = out_pool.tile([128, TOPK_OUT], f32, tag="vals")
                pos = out_pool.tile([128, TOPK_OUT], u32, tag="pos")
                for r in range(3):
                    vslice = vals[:, r * 8 : (r + 1) * 8]
                    nc.vector.max(out=vslice, in_=work[:, :])
                    nc.vector.max_index(
                        out=pos[:, r * 8 : (r + 1) * 8], in_max=vslice, in_values=work[:, :]
                    )
                    if r < 2:
                        nc.vector.match_replace(
                            out=work[:, :], in_to_replace=vslice,
                            in_values=work[:, :], imm_value=NEG,
                        )
                nc.sync.dma_start(out=out_vals[qt], in_=vals[:, :])
                nc.sync.dma_start(out=out_pos[qt], in_=pos[:, :])

    nc.compile()
    return nc


def _nseg_for(labels):
    return sum(-(-int((labels == c).sum()) // SEG) for c in range(NUM_CLASSES))


def _prep_core(tn, labels, nseg):
    """tn: [SHARD, D] fp32 normalized rows; labels: [SHARD] ints.
    Returns (padded [nseg*SEG, D] fp32, seg_label [nseg] int)."""
    order = np.argsort(labels, kind="stable")
    tn = tn[order]
    labels = labels[order]
    padded = np.zeros((nseg * SEG, D), dtype=np.float32)
    seg_label = np.zeros(nseg, dtype=np.int64)
    row = 0
    for c in range(NUM_CLASSES):
        blk = tn[labels == c]
        n = len(blk)
        if n == 0:
            continue
        padded[row : row + n] = blk
        nseg_c = -(-n // SEG)
        seg_label[row // SEG : row // SEG + nseg_c] = c
        row += nseg_c * SEG
    assert row <= nseg * SEG, f"padding overflow: {row}"
    return padded, seg_label


def _split_bf16(a):
    import ml_dtypes

    hi = a.astype(ml_dtypes.bfloat16)
    lo = (a - hi.astype(np.float32)).astype(ml_dtypes.bfloat16)
    return hi, lo


def _to_kdn(a_t):  # [N, D] -> [2, 128, N] (transposed, K-chunked)
    return np.ascontiguousarray(a_t.T.reshape(2, 128, -1))


def kernel(train_features, train_labels, x, k):
    from concourse.bass_utils import run_bass_kernel_spmd

    train_features = np.asarray(train_features, dtype=np.float32)
    x = np.asarray(x, dtype=np.float32)
    labels_np = np.asarray(train_labels).astype(np.int64)
    k = int(k)
    assert 0 < k <= TOPK_OUT, f"k={k} unsupported (device extracts {TOPK_OUT})" 

    norms = np.sqrt((train_features.astype(np.float32) ** 2).sum(axis=1, keepdims=True))
    tn = train_features / norms

    shard_labels = [labels_np[c * SHARD : (c + 1) * SHARD] for c in range(N_CORES)]
    nseg = max(_nseg_for(sl) for sl in shard_labels)
    seg_labels = []
    in_maps = []
    if MODE == "bf16x3":
        x_hi, x_lo = _split_bf16(x)
        x_hi_k, x_lo_k = _to_kdn(x_hi), _to_kdn(x_lo)
    else:
        x_k = _to_kdn(x)
    for c in range(N_CORES):
        sl = slice(c * SHARD, (c + 1) * SHARD)
        padded, seg_label = _prep_core(tn[sl], shard_labels[c], nseg)
        seg_labels.append(seg_label)
        if MODE == "bf16x3":
            t_hi, t_lo = _split_bf16(padded)
            in_maps.append({
                "t_hi": _to_kdn(t_hi), "t_lo": _to_kdn(t_lo),
                "x_hi": x_hi_k, "x_lo": x_lo_k,
            })
        else:
            in_maps.append({"t_full": _to_kdn(padded), "x_full": x_k})

    key = (MODE, nseg)
    if key not in _compiled:
        _compiled[key] = _build(MODE, nseg)
    nc = _compiled[key]

    res = run_bass_kernel_spmd(nc, in_maps, list(range(N_CORES))).results

    all_vals = np.concatenate(
        [res[c]["out_vals"].reshape(N_TEST, TOPK_OUT) for c in range(N_CORES)], axis=1
    )  # [N_TEST, 8*24]
    all_labs = np.concatenate(
        [
            seg_labels[c][res[c]["out_pos"].reshape(N_TEST, TOPK_OUT).astype(np.int64) // L1_KEEP]
            for c in range(N_CORES)
        ],
        axis=1,
    )

    sel = np.argpartition(-all_vals, k - 1, axis=1)[:, :k]
    votes = np.take_along_axis(all_labs, sel, axis=1)  # [N_TEST, K]
    counts = np.zeros((N_TEST, NUM_CLASSES), dtype=np.int32)
    for c in range(NUM_CLASSES):
        counts[:, c] = (votes == c).sum(axis=1)
    preds = counts.argmax(axis=1).astype(np.float32)
    return preds

